# revision 2
# baseline (speedup 1.0000x reference)
"""Correlation-network kernel v3 for TRN2, batch-sharded over 8 NeuronCores.

out[m, n] = (A @ B^T)[m, n] / sqrt(sum_m (A @ B^T)[m, n]^2)   per sample.

Structure (single-shot optimized; DMA is the binding resource at ~330 GB/s
per core, ~75us for 26MB in+out):
  - bf16 matmuls (PE issues a [128k,128m,512n] matmul+LDW every ~220ns).
  - Gram chain for column norms; s = pen2^-1/2 in one scalar-engine
    Abs_reciprocal_sqrt op (same act table as Copy), output straight to bf16.
  - Scale folded into B^T (r = bt * broadcast(s16)).
  - Early phase: first 3 m-tiles stream out chunk-wise while the remaining
    scale chunks pipeline (s-chain ci+1 and B-transposes ci+2 interleaved
    into the PE stream).
  - Main phase: m-tile-outer, full-row panels (sequential DRAM writes),
    panel DMAs alternating between the SP and Act HWDGE queues.
  - PSUM->SBUF drains alternate DVE/Act; gpsimd does the input casts.
"""
import numpy as np

B, H, W, C = 8, 48, 48, 256
HW = H * W            # 2304
MT = HW // 128        # 18 m-tiles
EARLY = 3             # m-tiles streamed chunk-wise during the s-pipeline
CHUNKS = [(0, 512), (512, 512), (1024, 512), (1536, 512), (2048, 256)]
_CACHE = {}


def _build(reps=1):
    import concourse.bacc as bacc
    import concourse.mybir as mybir
    import concourse.tile as tile
    from concourse.masks import make_identity

    dt = mybir.dt
    f32 = dt.float32
    bf = dt.bfloat16
    AF = mybir.ActivationFunctionType

    nc = bacc.Bacc(None, target_bir_lowering=False, debug=False)
    a_dram = nc.dram_tensor("a", [HW, C], f32, kind="ExternalInput")
    b_dram = nc.dram_tensor("b", [HW, C], f32, kind="ExternalInput")
    o_dram = nc.dram_tensor("out", [HW, HW], f32, kind="ExternalOutput")

    a_r = a_dram[:, :].rearrange("(t p) c -> p t c", p=128)
    b_r = b_dram[:, :].rearrange("(t p) c -> p t c", p=128)
    o_r = o_dram[:, :].rearrange("(t p) n -> p t n", p=128)

    with tile.TileContext(nc) as tc:
        sb = tc.alloc_tile_pool(name="sb", bufs=1)
        ps_t = tc.alloc_tile_pool(name="ps_t", bufs=1, space="PSUM")
        ps_m = tc.alloc_tile_pool(name="ps_m", bufs=1, space="PSUM")

        # ---- constants (once) ----
        id_f = sb.tile([128, 128], f32, name="idf")
        make_identity(nc, id_f)
        id_b = sb.tile([128, 128], bf, name="idb")
        nc.gpsimd.tensor_copy(id_b, id_f)
        ones_col_f = sb.tile([128, 1], f32, name="ocf")
        nc.gpsimd.memset(ones_col_f, 1.0)
        ones_col = sb.tile([128, 1], bf, name="oc")
        nc.gpsimd.tensor_copy(ones_col, ones_col_f)
        ones_row_f = sb.tile([1, 128], f32, name="orf")
        nc.gpsimd.memset(ones_row_f, 1.0)
        ones_row = sb.tile([1, 128], bf, name="orow")
        nc.gpsimd.tensor_copy(ones_row, ones_row_f)

        for _rep in range(reps):
            a_f = sb.tile([128, MT, 256], f32, tag="af", name="af")
            b_f = sb.tile([128, MT, 256], f32, tag="bfi", name="bfi")
            a16 = sb.tile([128, MT, 256], bf, tag="a16", name="a16")
            b16 = sb.tile([128, MT, 256], bf, tag="b16", name="b16")
            at = [sb.tile([128, HW], bf, tag=f"at{h}", name=f"at{h}", bufs=2)
                  for h in (0, 1)]
            bt = [sb.tile([128, HW], bf, tag=f"bt{h}", name=f"bt{h}", bufs=2)
                  for h in (0, 1)]
            r = [sb.tile([128, HW], bf, tag=f"r{h}", name=f"r{h}", bufs=2)
                 for h in (0, 1)]
            g16 = [sb.tile([128, 256], bf, tag=f"g{h}", name=f"g{h}", bufs=2)
                   for h in (0, 1)]
            s16 = sb.tile([1, HW], bf, tag="s16", name="s16", bufs=2)

            # ---- input DMAs (SP queue; A first after tiny B0) ----
            nc.sync.dma_start(out=b_f[:, 0:4], in_=b_r[:, 0:4])
            nc.sync.dma_start(out=a_f[:, 0:6], in_=a_r[:, 0:6])
            nc.sync.dma_start(out=a_f[:, 6:12], in_=a_r[:, 6:12])
            nc.sync.dma_start(out=a_f[:, 12:18], in_=a_r[:, 12:18])
            nc.sync.dma_start(out=b_f[:, 4:8], in_=b_r[:, 4:8])
            nc.sync.dma_start(out=b_f[:, 8:12], in_=b_r[:, 8:12])
            nc.sync.dma_start(out=b_f[:, 12:16], in_=b_r[:, 12:16])
            nc.sync.dma_start(out=b_f[:, 16:18], in_=b_r[:, 16:18])

            # ---- casts (A on Act, B0 on DVE, B-rest on Act/Pool) ----
            nc.vector.tensor_copy(b16[:, 0:4], b_f[:, 0:4])

            # ---- PE helpers ----
            def bt_batch(ci):
                n0, cw = CHUNKS[ci]
                t0, t1 = 4 * ci, min(4 * ci + 4, MT)
                for h in (0, 1):
                    pt = ps_t.tile([128, 512], bf, tag="pt", name="pt",
                                   bufs=1)
                    for t in range(t0, t1):
                        nc.tensor.transpose(
                            pt[:, (t - t0) * 128:(t - t0 + 1) * 128],
                            b16[:, t, h * 128:(h + 1) * 128], id_b)
                    nc.vector.tensor_copy(bt[h][:, n0:n0 + cw], pt[:, :cw])

            def at_batch(blk):
                for half in (0, 1):
                    t0 = 6 * blk + 3 * half
                    for h in (0, 1):
                        pt = ps_t.tile([128, 512], bf, tag="pt", name="pt",
                                       bufs=1)
                        for t in range(t0, t0 + 3):
                            nc.tensor.transpose(
                                pt[:, (t - t0) * 128:(t - t0 + 1) * 128],
                                a16[:, t, h * 128:(h + 1) * 128], id_b)
                        nc.vector.tensor_copy(
                            at[h][:, t0 * 128:(t0 + 3) * 128], pt[:, :384])

            # ---- gram + A transposes, interleaved per cast block ----
            pg = [ps_t.tile([128, 512], f32, tag="pgq", name=f"pg{h}",
                            bufs=2) for h in (0, 1)]
            for blk in range(3):
                if blk == 0:
                    bt_batch(0)
                nc.scalar.copy(a16[:, 6 * blk:6 * blk + 6],
                               a_f[:, 6 * blk:6 * blk + 6])
                for h in (0, 1):
                    for t in range(6 * blk, 6 * blk + 6):
                        nc.tensor.matmul(
                            pg[h][:, :256], a16[:, t, h * 128:(h + 1) * 128],
                            a16[:, t, :], start=(t == 0), stop=(t == MT - 1))
                at_batch(blk)
            for h in (0, 1):
                nc.scalar.copy(g16[h], pg[h][:, :256])
            nc.gpsimd.tensor_copy(b16[:, 4:8], b_f[:, 4:8])
            nc.gpsimd.tensor_copy(b16[:, 8:12], b_f[:, 8:12])
            nc.gpsimd.tensor_copy(b16[:, 12:16], b_f[:, 12:16])
            nc.gpsimd.tensor_copy(b16[:, 16:18], b_f[:, 16:18])

            # ---- s-chain ----
            def s_pq(ci):
                n0, cw = CHUNKS[ci]
                for h2 in (0, 1):
                    pq = ps_t.tile([128, 512], f32, tag="pgq",
                                   name=f"pq{h2}", bufs=2)
                    for h in (0, 1):
                        nc.tensor.matmul(
                            pq[:, :cw],
                            g16[h][:, h2 * 128:(h2 + 1) * 128],
                            bt[h][:, n0:n0 + cw],
                            start=(h == 0), stop=(h == 1))
                    nc.vector.tensor_mul(r[h2][:, n0:n0 + cw],
                                         bt[h2][:, n0:n0 + cw],
                                         pq[:, :cw])

            def s_pp(ci):
                n0, cw = CHUNKS[ci]
                pp = ps_m.tile([128, 512], f32, tag="sch", name="pp", bufs=2)
                for h2 in (0, 1):
                    nc.tensor.matmul(pp[0:1, :cw], ones_col,
                                     r[h2][:, n0:n0 + cw],
                                     start=(h2 == 0), stop=(h2 == 1))
                nc.scalar.activation(s16[:, n0:n0 + cw], pp[0:1, :cw],
                                     AF.Abs_reciprocal_sqrt)

            def s_part2(ci):
                n0, cw = CHUNKS[ci]
                pb = ps_m.tile([128, 512], f32, tag="sch", name="pb", bufs=2)
                nc.tensor.matmul(pb[:, :cw], ones_row, s16[0:1, n0:n0 + cw],
                                 start=True, stop=True)
                for h in (0, 1):
                    nc.vector.tensor_mul(r[h][:, n0:n0 + cw],
                                         bt[h][:, n0:n0 + cw], pb[:, :cw])

            qcnt = [0]

            def out_dma(dst, src):
                eng = nc.sync if qcnt[0] % 2 == 0 else nc.scalar
                eng.dma_start(out=dst, in_=src)
                qcnt[0] += 1

            # ---- s-chain pipeline with early m-tiles as PE filler ----

            dcnt = [0]

            def pair_drain(mt, ci, dst):
                n0, cw = CHUNKS[ci]
                pm = ps_m.tile([128, 512], f32, tag="pm", name="pm", bufs=3)
                for h in (0, 1):
                    nc.tensor.matmul(
                        pm[:, :cw], at[h][:, mt * 128:(mt + 1) * 128],
                        r[h][:, n0:n0 + cw], start=(h == 0), stop=(h == 1))
                cp = (nc.vector.tensor_copy if dcnt[0] % 2 == 0
                      else nc.scalar.copy)
                cp(dst, pm[:, :cw])
                dcnt[0] += 1

            def early_pairs(ci):
                n0, cw = CHUNKS[ci]
                for mt in range(EARLY):
                    ob = sb.tile([128, 512], f32, tag="ob", name="ob",
                                 bufs=6)
                    pair_drain(mt, ci, ob[:, :cw])
                    out_dma(o_r[:, mt, n0:n0 + cw], ob[:, :cw])

            NC = len(CHUNKS)
            for ci in range(NC):
                s_pq(ci)
                if ci + 1 < NC:
                    bt_batch(ci + 1)
                if ci >= 1:
                    s_part2(ci - 1)
                if ci >= 2:
                    early_pairs(ci - 2)
                s_pp(ci)
            s_part2(NC - 1)
            early_pairs(NC - 2)
            early_pairs(NC - 1)

            # ---- main phase: full panels ----
            for mt in range(EARLY, MT):
                panel = sb.tile([128, HW], f32, tag="panel", name="panel",
                                bufs=5)
                for ci, (n0, cw) in enumerate(CHUNKS):
                    pair_drain(mt, ci, panel[:, n0:n0 + cw])
                out_dma(o_r[:, mt, :], panel)

        ps_m.release()
        ps_t.release()
        sb.release()
    nc.finalize()
    return nc


def _get_nc(reps=1):
    key = ("nc", reps)
    if key not in _CACHE:
        _CACHE[key] = _build(reps)
    return _CACHE[key]


def run(feature_A, feature_B, trace=False):
    from concourse.bass_utils import run_bass_kernel_spmd

    nc = _get_nc()
    fa = np.ascontiguousarray(np.asarray(feature_A), dtype=np.float32)
    fb = np.ascontiguousarray(np.asarray(feature_B), dtype=np.float32)
    in_maps = [{"a": fa[i].reshape(HW, C), "b": fb[i].reshape(HW, C)}
               for i in range(B)]
    res = run_bass_kernel_spmd(nc, in_maps, list(range(B)), trace=trace)
    out = np.stack([res.results[i]["out"] for i in range(B)])
    return out.reshape(B, H, W, H, W), res


def kernel(feature_A, feature_B):
    out, _ = run(feature_A, feature_B)
    return out
